# revision 15
# baseline (speedup 1.0000x reference)
"""GQA attention with QK-norm for Trainium2, sharded over 8 NeuronCores.

Problem: B=2, T=2048, D=2048, H=16 query heads, KVH=4 kv heads, dk=128.
    Q = q @ Wq.T ; K = k @ Wk.T ; V = v @ Wv.T  (per batch)
    Q = g * l2norm(Q, per head) ; K = l2norm(K, per head)
    out = softmax(causal(Q K^T / sqrt(dk))) V @ Wo.T

Sharding: core c = 4*b + gi handles batch b and kv-head group gi
(4 query heads + 1 kv head). Each core computes a row-shard of the
output projection; the host sums the 4 partials per batch.

Key structure (v3):
- Three DMA queues: sync carries wq + the q-activation stream, scalar
  carries the kT stream, gpsimd carries a fully prefetched vT buffer
  + wo.  Q-proj (tb-major) interleaves with K-proj quarters so both
  streams are consumed at their arrival pace; V-proj runs last out of
  SBUF and covers the K-norm chains.
- All reciprocal-type math on the Scalar engine in ONE table set
  (natural_log_exp_and_others): softmax inv = Exp(-Ln(rowsum)); norm
  scale = Exp(-0.5*Ln(sum x^2) + ln(g/sqrt(dk))).  Norm chains run
  inline per (head, tb) hidden behind later matmuls.
- [1,512] rows are partition-broadcast with a K=1 PE outer product
  (ones[1,128].T @ row), never via DMA round trips.
- Attention: per (h, q-block), k-tiles are processed diagonal-first;
  rowsum/Y matmuls for pair p-1 are emitted between S-matmul pairs p
  (software pipeline) so the PE never waits on exp; causal masking is
  a gpsimd affine_select zero-fill on the exp-ed strip; diagonal
  k-tiles only stream live q>=p+128j columns.
"""

import math
import sys

for _p in ("/opt/trn_rl_repo",):
    if _p not in sys.path:
        sys.path.append(_p)

import numpy as np
from concourse import bacc, mybir, tile
from concourse.bass_utils import run_bass_kernel_spmd

B, T, D, H, KVH, DK = 2, 2048, 2048, 16, 4, 128
HPG = H // KVH          # query heads per core (group)
E = HPG * DK            # 512: q-head dims per core
P = 128
TB = 4                  # t blocks of 512
NT = T // P             # 16 tiles of 128 along T
ND = D // P             # 16 contraction tiles
f32 = mybir.dt.float32
bf16 = mybir.dt.bfloat16
AF = mybir.ActivationFunctionType


def _single_act_table(fn):
    """Run fn with the ACT-table chooser forced to resolve every
    activation into natural_log_exp_and_others (contains Ln, Exp,
    Copy, Identity, Square -- everything this kernel uses).  The
    default chooser puts Exp and Ln in different sets and reloads
    tables (~1.3us) on every alternation.  Set ids stay valid: the
    dict keeps its length and insertion order; only the other sets'
    contents are hidden while our compile runs."""
    import concourse.bacc as _bm
    keep = "natural_log_exp_and_others"
    orig = _bm.get_activation_tables

    def patched(arch):
        t = orig(arch)
        return {name: (set(funcs) if name == keep else set())
                for name, funcs in t.items()}

    _bm.get_activation_tables = patched
    try:
        return fn()
    finally:
        _bm.get_activation_tables = orig


def build_kernel():
    nc = bacc.Bacc(None, target_bir_lowering=False)

    qTt = nc.declare_dram_parameter("qTt", [TB, P, ND * 512], bf16,
                                    isOutput=False)
    kT = nc.declare_dram_parameter("kT", [D, T], bf16, isOutput=False)
    vT = nc.declare_dram_parameter("vT", [D, T], bf16, isOutput=False)
    wqt = nc.declare_dram_parameter("wqt", [P, ND * E], bf16, isOutput=False)
    wkt = nc.declare_dram_parameter("wkt", [P, ND * DK], bf16, isOutput=False)
    wvt = nc.declare_dram_parameter("wvt", [P, ND * DK], bf16, isOutput=False)
    wot = nc.declare_dram_parameter("wot", [P, HPG * D], bf16, isOutput=False)
    lng = nc.declare_dram_parameter("lng", [1, HPG], f32, isOutput=False)
    outT = nc.declare_dram_parameter("outT", [D, T], f32, isOutput=True)

    from contextlib import ExitStack

    with tile.TileContext(nc) as tc:
        with ExitStack() as outer:
            const = outer.enter_context(tc.tile_pool(name="const", bufs=1))
            persist = outer.enter_context(tc.tile_pool(name="persist", bufs=1))

            ones_f32 = const.tile([P, 1], f32, tag="ones_f32")
            nc.vector.memset(ones_f32[:], 1.0)
            ones_col = const.tile([P, 1], bf16, tag="ones_col")
            nc.vector.tensor_copy(ones_col[:], ones_f32[:])
            onesr_f32 = const.tile([1, P], f32, tag="onesr_f32")
            nc.vector.memset(onesr_f32[:], 1.0)
            ones_row = const.tile([1, P], bf16, tag="ones_row")
            nc.vector.tensor_copy(ones_row[:], onesr_f32[:])
            lng_sb = const.tile([1, HPG], f32, tag="lng")
            nc.sync.dma_start(lng_sb[:], lng[:])

            qt_sb = persist.tile([P, HPG * T], bf16, tag="qt")
            kt_sb = persist.tile([P, T], bf16, tag="kt")
            vtm_sb = persist.tile([P, T], bf16, tag="vtm")
            yt_sb = persist.tile([P, HPG * T], bf16, tag="yt")

            # ---------------- phase A: projections + norms ----------------
            with ExitStack() as pa:
                wpool = pa.enter_context(tc.tile_pool(name="wpool", bufs=1))
                vbufp = pa.enter_context(tc.tile_pool(name="vbuf", bufs=1))
                qstream = pa.enter_context(tc.tile_pool(name="qstream",
                                                        bufs=6))
                kroll = pa.enter_context(tc.tile_pool(name="kroll", bufs=6))
                vstage = pa.enter_context(tc.tile_pool(name="vstage", bufs=1))
                sqpool = pa.enter_context(tc.tile_pool(name="sqpool", bufs=2))
                rows = pa.enter_context(tc.tile_pool(name="rows", bufs=4))
                psA = pa.enter_context(
                    tc.tile_pool(name="psA", bufs=6, space="PSUM"))
                psM = pa.enter_context(
                    tc.tile_pool(name="psM", bufs=2, space="PSUM"))

                # DMA layout: sync queue carries wq + the q-activation
                # stream (interleaved in first-need order), scalar
                # queue carries wk + the kT stream, gpsimd queue
                # carries vT + wo -- but gated behind the arrival of
                # tb0's q chunks so the prefetch doesn't starve the
                # startup-critical stream.
                wq_sb = wpool.tile([P, ND * E], bf16, tag="wq")
                qch0 = []
                for c in range(4):
                    nc.sync.dma_start(
                        wq_sb[:, c * 2048:(c + 1) * 2048],
                        wqt[:, c * 2048:(c + 1) * 2048])
                    a = qstream.tile([P, 2048], bf16, tag="qch",
                                     name=f"qch0_{c}")
                    nc.sync.dma_start(a[:], qTt[0, :, c * 2048:(c + 1) * 2048])
                    qch0.append(a)
                wk_sb = wpool.tile([P, ND * DK], bf16, tag="wk")
                nc.scalar.dma_start(wk_sb[:], wkt[:])
                wv_sb = wpool.tile([P, ND * DK], bf16, tag="wv")
                nc.sync.dma_start(wv_sb[:], wvt[:])
                vbuf = vbufp.tile([P, ND * T], bf16, tag="vbuf")
                vgate = const.tile([1, 64], bf16, tag="vgate")
                nc.gpsimd.tensor_copy(vgate[:], qch0[3][0:1, 0:64])
                for n in range(ND):
                    nc.gpsimd.dma_start(
                        vbuf[:, n * T:(n + 1) * T],
                        vT[n * P:(n + 1) * P, :])

                deferred = []

                def flush(upto=None):
                    n = len(deferred) if upto is None else upto
                    for _ in range(n):
                        deferred.pop(0)()

                def norm_chain(xs_full, bias):
                    """Queue the scale-application for columns of
                    xs_full [128, 512] by e^bias/||col||.  Emits the
                    rowsum + ACT chain now; the PE broadcast + DVE
                    multiply are deferred so later matmuls can cover
                    the ACT latency."""
                    sq = sqpool.tile([P, 512], bf16, tag="sq")
                    nc.vector.tensor_mul(sq[:], xs_full, xs_full)
                    pm = psM.tile([P, 512], f32, tag="pm")
                    nc.tensor.matmul(pm[0:1, :], ones_col[:], sq[:],
                                     start=True, stop=True)
                    u = rows.tile([1, 512], f32, tag="u")
                    nc.scalar.activation(u[:], pm[0:1, :], AF.Ln)
                    iv = rows.tile([1, 512], bf16, tag="iv")
                    nc.scalar.activation(iv[:], u[:], AF.Exp,
                                         bias=bias, scale=-0.5)

                    def apply():
                        pb = psM.tile([P, 512], f32, tag="pm")
                        nc.tensor.matmul(pb[:], ones_row[:], iv[:],
                                         start=True, stop=True)
                        nc.vector.tensor_mul(xs_full, xs_full, pb[:])
                    deferred.append(apply)

                # K accumulators live across the whole interleaved loop.
                kaccs = [psA.tile([P, 512], f32, tag="acc",
                                  name=f"kacc{_t}") for _t in range(TB)]

                # Q-proj tb-major, K-proj quarters interleaved.
                for tb in range(TB):
                    if tb == 0:
                        qch = qch0
                    else:
                        qch = []
                        for c in range(4):
                            a = qstream.tile([P, 2048], bf16, tag="qch")
                            nc.sync.dma_start(
                                a[:], qTt[tb, :, c * 2048:(c + 1) * 2048])
                            qch.append(a)
                    for h in range(HPG):
                        ps = psA.tile([P, 512], f32, tag="acc")
                        for n in range(ND):
                            nc.tensor.matmul(
                                ps[:],
                                wq_sb[:, n * E + h * P:n * E + (h + 1) * P],
                                qch[n // 4][:, (n % 4) * 512:
                                            (n % 4 + 1) * 512],
                                start=(n == 0), stop=(n == ND - 1))
                        flush()
                        xs = qt_sb[:, h * T + tb * 512:h * T + (tb + 1) * 512]
                        nc.vector.tensor_copy(xs, ps[:])
                        norm_chain(xs, lng_sb[0:1, h:h + 1])
                    # K quarter: 4 kT slices stream in, 16 matmuls.
                    for n in range(4 * tb, 4 * tb + 4):
                        a = kroll.tile([P, T], bf16, tag="kch")
                        nc.scalar.dma_start(a[:], kT[n * P:(n + 1) * P, :])
                        for t2 in range(TB):
                            nc.tensor.matmul(
                                kaccs[t2][:],
                                wk_sb[:, n * DK:(n + 1) * DK],
                                a[:, t2 * 512:(t2 + 1) * 512],
                                start=(n == 0), stop=(n == ND - 1))
                flush()

                # K copies + norm chains; V-proj provides PE cover.
                for t2 in range(TB):
                    xs = kt_sb[:, t2 * 512:(t2 + 1) * 512]
                    nc.vector.tensor_copy(xs, kaccs[t2][:])
                    norm_chain(xs, 0.0)

                vaccs = [psA.tile([P, 512], f32, tag="acc",
                                  name=f"vacc{_t}") for _t in range(TB)]
                for n in range(ND):
                    for t2 in range(TB):
                        nc.tensor.matmul(
                            vaccs[t2][:],
                            wv_sb[:, n * DK:(n + 1) * DK],
                            vbuf[:, n * T + t2 * 512:n * T + (t2 + 1) * 512],
                            start=(n == 0), stop=(n == ND - 1))
                    if n == 5:
                        flush()  # K bc-MMs + muls under V-proj cover
                vt_stage = vstage.tile([P, T], bf16, tag="vst")
                for t2 in range(TB):
                    nc.any.tensor_copy(
                        vt_stage[:, t2 * 512:(t2 + 1) * 512], vaccs[t2][:])
                # per-128-block transposes V^T -> V via the DMA XBAR
                # (SBUF->SBUF, off the PE entirely)
                for n in range(NT):
                    nc.sync.dma_start(
                        vtm_sb[:, n * P:(n + 1) * P],
                        vt_stage[:, n * P:(n + 1) * P], transpose=True)

            # ------------- phase B+C: attention + out projection ----------
            atp = outer.enter_context(tc.tile_pool(name="atp", bufs=3))
            rows2 = outer.enter_context(tc.tile_pool(name="rows2", bufs=4))
            wo_pool = outer.enter_context(tc.tile_pool(name="wo", bufs=1))
            ostage = outer.enter_context(tc.tile_pool(name="ostage", bufs=3))
            ps_st = outer.enter_context(
                tc.tile_pool(name="ps_st", bufs=2, space="PSUM"))
            ps_y = outer.enter_context(
                tc.tile_pool(name="ps_y", bufs=1, space="PSUM"))
            ps_misc = outer.enter_context(
                tc.tile_pool(name="ps_misc", bufs=1, space="PSUM"))
            ps_o = outer.enter_context(
                tc.tile_pool(name="ps_o", bufs=2, space="PSUM"))

            wo_sb = wo_pool.tile([P, HPG * D], bf16, tag="wo")
            nc.gpsimd.dma_start(wo_sb[:], wot[:])

            deferredB = []

            def flushB():
                while deferredB:
                    deferredB.pop(0)()

            for qb in range(TB):
                n_k = 4 * (qb + 1)
                # diagonal k-tiles first so the head's last exp has no
                # gpsimd select behind it; pairs stay j-aligned.
                ktiles = list(range(4 * qb, 4 * qb + 4)) + list(range(4 * qb))
                pairs = [ktiles[2 * i:2 * i + 2] for i in range(n_k // 2)]
                for h in range(HPG):
                    qh = qt_sb[:, h * T + qb * 512:h * T + (qb + 1) * 512]
                    strip = atp.tile([P, NT * 512], bf16, tag="strip")
                    # pm/ps_yt are allocated lazily at first use: their
                    # 1-buf pools are shared with the previous head's
                    # deferred broadcast tile, so allocation order must
                    # match PE emission order.
                    state = {}

                    def rowsum_y(pi, first, last, state=state):
                        if "pm" not in state:
                            state["pm"] = ps_misc.tile(
                                [P, 512], f32, tag="pm", name="pm")
                            state["y"] = ps_y.tile(
                                [P, 512], f32, tag="y", name="psyt")
                        pm, ps_yt = state["pm"], state["y"]
                        for kt in pairs[pi]:
                            j = kt - 4 * qb
                            off = 128 * j if j > 0 else 0
                            sl = strip[:, kt * 512 + off:(kt + 1) * 512]
                            nc.tensor.matmul(
                                pm[0:1, off:512], ones_col[:], sl,
                                start=(first and kt == pairs[pi][0]),
                                stop=(last and kt == pairs[pi][1]))
                        for kt in pairs[pi]:
                            j = kt - 4 * qb
                            off = 128 * j if j > 0 else 0
                            sl = strip[:, kt * 512 + off:(kt + 1) * 512]
                            nc.tensor.matmul(
                                ps_yt[:, off:512],
                                vtm_sb[:, kt * P:(kt + 1) * P], sl,
                                start=(first and kt == pairs[pi][0]),
                                stop=(last and kt == pairs[pi][1]))

                    for pi, pr in enumerate(pairs):
                        st = ps_st.tile([P, 1024], f32, tag="st")
                        if pi == 0:
                            flushB()  # prev head's bc-MM + scale
                        for j2, kt in enumerate(pr):
                            j = kt - 4 * qb
                            off = 128 * j if j > 0 else 0
                            nc.tensor.matmul(
                                st[:, j2 * 512 + off:(j2 + 1) * 512],
                                kt_sb[:, kt * P:(kt + 1) * P],
                                qh[:, off:512], start=True, stop=True)
                        ssl = strip[:, pr[0] * 512:pr[0] * 512 + 1024]
                        nc.scalar.activation(ssl, st[:], AF.Exp)
                        j0 = pr[0] - 4 * qb
                        if j0 >= 0:  # diagonal pair: causal zero-fill
                            nc.gpsimd.affine_select(
                                out=ssl, in_=ssl,
                                compare_op=mybir.AluOpType.is_ge,
                                fill=0.0, base=-128 * j0,
                                pattern=[[-128, 2], [1, 512]],
                                channel_multiplier=-1,
                            )
                        if pi > 0:
                            rowsum_y(pi - 1, first=(pi == 1), last=False)
                    rowsum_y(len(pairs) - 1, first=(len(pairs) == 1),
                             last=True)
                    pm, ps_yt = state["pm"], state["y"]
                    u = rows2.tile([1, 512], f32, tag="u")
                    nc.scalar.activation(u[:], pm[0:1, :], AF.Ln)
                    iv = rows2.tile([1, 512], bf16, tag="iv")
                    nc.scalar.activation(iv[:], u[:], AF.Exp, scale=-1.0)
                    yslice = yt_sb[:, h * T + qb * 512:h * T + (qb + 1) * 512]
                    nc.vector.tensor_copy(yslice, ps_yt[:])

                    def scale_y(iv=iv, yslice=yslice):
                        pb = ps_misc.tile([P, 512], f32, tag="pm")
                        nc.tensor.matmul(pb[:], ones_row[:], iv[:],
                                         start=True, stop=True)
                        nc.vector.tensor_mul(yslice, yslice, pb[:])
                    deferredB.append(scale_y)
                flushB()

                # out projection for this t-block (overlaps next q-block)
                tb = qb
                for ot in range(NT):
                    ps = ps_o.tile([P, 512], f32, tag="o")
                    for h in range(HPG):
                        nc.tensor.matmul(
                            ps[:],
                            wo_sb[:, h * D + ot * P:h * D + (ot + 1) * P],
                            yt_sb[:, h * T + tb * 512:h * T + (tb + 1) * 512],
                            start=(h == 0), stop=(h == HPG - 1))
                    o_sb = ostage.tile([P, 512], f32, tag="osb")
                    nc.any.tensor_copy(o_sb[:], ps[:])
                    nc.sync.dma_start(
                        outT[ot * P:(ot + 1) * P, tb * 512:(tb + 1) * 512],
                        o_sb[:])

    _single_act_table(nc.compile)
    return nc


def make_in_maps(q, k, v, Wq, Wk, Wv, Wo, g):
    import ml_dtypes
    st = ml_dtypes.bfloat16
    in_maps = []
    act_t = {}
    for b in range(B):
        qTb = np.ascontiguousarray(q[b].T).astype(st)
        # [TB, P, ND*512]: row p of block tb = concat_n qT[n*128+p, tb*512:]
        qTt = np.ascontiguousarray(
            qTb.reshape(ND, P, TB, 512).transpose(2, 1, 0, 3)
            .reshape(TB, P, ND * 512))
        act_t[b] = (
            qTt,
            np.ascontiguousarray(k[b].T).astype(st),
            np.ascontiguousarray(v[b].T).astype(st),
        )

    def wtile(wT, cols):  # wT: (D, cols) -> [P, ND*cols] row-tiled
        return np.ascontiguousarray(
            np.ascontiguousarray(wT).reshape(-1, P, cols)
            .transpose(1, 0, 2).reshape(P, -1)).astype(st)

    g_flat = np.asarray(g, dtype=np.float32).reshape(H)
    for c in range(8):
        b, gi = divmod(c, KVH)
        qTt, kTb, vTb = act_t[b]
        e0 = gi * E
        gvals = g_flat[gi * HPG:(gi + 1) * HPG] / math.sqrt(DK)
        in_maps.append({
            "qTt": qTt, "kT": kTb, "vT": vTb,
            "wqt": wtile(Wq[e0:e0 + E, :].T, E),
            "wkt": wtile(Wk[gi * DK:(gi + 1) * DK, :].T, DK),
            "wvt": wtile(Wv[gi * DK:(gi + 1) * DK, :].T, DK),
            "wot": wtile(Wo[:, e0:e0 + E].T, D),
            "lng": np.log(gvals)[None, :].astype(np.float32),
        })
    return in_maps


_cached = {}


def kernel(q, k, v, Wq, Wk, Wv, Wo, g, _trace=False, _tmpdir=None):
    if "nc" not in _cached:
        _cached["nc"] = build_kernel()
    nc = _cached["nc"]
    in_maps = make_in_maps(
        np.asarray(q, np.float32), np.asarray(k, np.float32),
        np.asarray(v, np.float32), np.asarray(Wq, np.float32),
        np.asarray(Wk, np.float32), np.asarray(Wv, np.float32),
        np.asarray(Wo, np.float32), g)
    res = run_bass_kernel_spmd(
        nc, in_maps, list(range(8)), trace=_trace, tmpdir=_tmpdir)
    out = np.empty((B, T, D), dtype=np.float32)
    for b in range(B):
        acc = res.results[4 * b]["outT"].copy()
        for gi in range(1, KVH):
            acc += res.results[4 * b + gi]["outT"]
        out[b] = acc.T
    kernel.last_results = res
    return out


# revision 16
# speedup vs baseline: 1.1007x; 1.1007x over previous
"""GQA attention with QK-norm for Trainium2, sharded over 8 NeuronCores.

Problem: B=2, T=2048, D=2048, H=16 query heads, KVH=4 kv heads, dk=128.
    Q = q @ Wq.T ; K = k @ Wk.T ; V = v @ Wv.T  (per batch)
    Q = g * l2norm(Q, per head) ; K = l2norm(K, per head)
    out = softmax(causal(Q K^T / sqrt(dk))) V @ Wo.T

Sharding: core c = 4*b + gi handles batch b and kv-head group gi
(4 query heads + 1 kv head). Each core computes a row-shard of the
output projection; the host sums the 4 partials per batch.

Key structure (v3):
- Three DMA queues: sync carries wq + the q-activation stream, scalar
  carries the kT stream, gpsimd carries a fully prefetched vT buffer
  + wo.  Q-proj (tb-major) interleaves with K-proj quarters so both
  streams are consumed at their arrival pace; V-proj runs last out of
  SBUF and covers the K-norm chains.
- All reciprocal-type math on the Scalar engine in ONE table set
  (natural_log_exp_and_others): softmax inv = Exp(-Ln(rowsum)); norm
  scale = Exp(-0.5*Ln(sum x^2) + ln(g/sqrt(dk))).  Norm chains run
  inline per (head, tb) hidden behind later matmuls.
- [1,512] rows are partition-broadcast with a K=1 PE outer product
  (ones[1,128].T @ row), never via DMA round trips.
- Attention: per (h, q-block), k-tiles are processed diagonal-first;
  rowsum/Y matmuls for pair p-1 are emitted between S-matmul pairs p
  (software pipeline) so the PE never waits on exp; causal masking is
  a gpsimd affine_select zero-fill on the exp-ed strip; diagonal
  k-tiles only stream live q>=p+128j columns.
"""

import math
import sys

for _p in ("/opt/trn_rl_repo",):
    if _p not in sys.path:
        sys.path.append(_p)

import numpy as np
from concourse import bacc, mybir, tile
from concourse.bass_utils import run_bass_kernel_spmd

B, T, D, H, KVH, DK = 2, 2048, 2048, 16, 4, 128
HPG = H // KVH          # query heads per core (group)
E = HPG * DK            # 512: q-head dims per core
P = 128
TB = 4                  # t blocks of 512
NT = T // P             # 16 tiles of 128 along T
ND = D // P             # 16 contraction tiles
f32 = mybir.dt.float32
bf16 = mybir.dt.bfloat16
AF = mybir.ActivationFunctionType


def _single_act_table(fn):
    """Run fn with the ACT-table chooser forced to resolve every
    activation into natural_log_exp_and_others (contains Ln, Exp,
    Copy, Identity, Square -- everything this kernel uses).  The
    default chooser puts Exp and Ln in different sets and reloads
    tables (~1.3us) on every alternation.  Set ids stay valid: the
    dict keeps its length and insertion order; only the other sets'
    contents are hidden while our compile runs."""
    import concourse.bacc as _bm
    keep = "natural_log_exp_and_others"
    orig = _bm.get_activation_tables

    def patched(arch):
        t = orig(arch)
        return {name: (set(funcs) if name == keep else set())
                for name, funcs in t.items()}

    _bm.get_activation_tables = patched
    try:
        return fn()
    finally:
        _bm.get_activation_tables = orig


def build_kernel():
    nc = bacc.Bacc(None, target_bir_lowering=False)

    qTt = nc.declare_dram_parameter("qTt", [TB, P, ND * 512], bf16,
                                    isOutput=False)
    kT = nc.declare_dram_parameter("kT", [D, T], bf16, isOutput=False)
    vT = nc.declare_dram_parameter("vT", [D, T], bf16, isOutput=False)
    wqt = nc.declare_dram_parameter("wqt", [P, ND * E], bf16, isOutput=False)
    wkt = nc.declare_dram_parameter("wkt", [P, ND * DK], bf16, isOutput=False)
    wvt = nc.declare_dram_parameter("wvt", [P, ND * DK], bf16, isOutput=False)
    wot = nc.declare_dram_parameter("wot", [P, HPG * D], bf16, isOutput=False)
    lng = nc.declare_dram_parameter("lng", [1, HPG], f32, isOutput=False)
    outT = nc.declare_dram_parameter("outT", [D, T], f32, isOutput=True)

    from contextlib import ExitStack

    with tile.TileContext(nc) as tc:
        with ExitStack() as outer:
            const = outer.enter_context(tc.tile_pool(name="const", bufs=1))
            persist = outer.enter_context(tc.tile_pool(name="persist", bufs=1))

            ones_f32 = const.tile([P, 1], f32, tag="ones_f32")
            nc.vector.memset(ones_f32[:], 1.0)
            ones_col = const.tile([P, 1], bf16, tag="ones_col")
            nc.vector.tensor_copy(ones_col[:], ones_f32[:])
            onesr_f32 = const.tile([1, P], f32, tag="onesr_f32")
            nc.vector.memset(onesr_f32[:], 1.0)
            ones_row = const.tile([1, P], bf16, tag="ones_row")
            nc.vector.tensor_copy(ones_row[:], onesr_f32[:])
            lng_sb = const.tile([1, HPG], f32, tag="lng")
            nc.sync.dma_start(lng_sb[:], lng[:])

            qt_sb = persist.tile([P, HPG * T], bf16, tag="qt")
            kt_sb = persist.tile([P, T], bf16, tag="kt")
            vtm_sb = persist.tile([P, T], bf16, tag="vtm")
            yt_sb = persist.tile([P, HPG * T], bf16, tag="yt")

            # ---------------- phase A: projections + norms ----------------
            with ExitStack() as pa:
                wpool = pa.enter_context(tc.tile_pool(name="wpool", bufs=1))
                vbufp = pa.enter_context(tc.tile_pool(name="vbuf", bufs=1))
                qstream = pa.enter_context(tc.tile_pool(name="qstream",
                                                        bufs=6))
                kroll = pa.enter_context(tc.tile_pool(name="kroll", bufs=6))
                vstage = pa.enter_context(tc.tile_pool(name="vstage", bufs=1))
                sqpool = pa.enter_context(tc.tile_pool(name="sqpool", bufs=2))
                rows = pa.enter_context(tc.tile_pool(name="rows", bufs=4))
                psA = pa.enter_context(
                    tc.tile_pool(name="psA", bufs=6, space="PSUM"))
                psM = pa.enter_context(
                    tc.tile_pool(name="psM", bufs=2, space="PSUM"))

                # DMA queue map (ordered by first need): the Q stream
                # (wq + q-activation chunks, 10MB) is split across BOTH
                # hardware DGE queues -- sync takes even chunks, scalar
                # takes odd chunks -- so it gets ~2/3 of HBM bandwidth
                # at startup.  kT rides the gpsimd SWDGE queue, paced
                # by the kroll pool.  vT/wo dispatches are gated below
                # with real data deps so the scheduler cannot hoist
                # them into the startup window.
                wq_sb = wpool.tile([P, ND * E], bf16, tag="wq")
                qch0 = []
                for c in range(4):
                    nc.sync.dma_start(
                        wq_sb[:, c * 2048:(c + 1) * 2048],
                        wqt[:, c * 2048:(c + 1) * 2048])
                    a = qstream.tile([P, 2048], bf16, tag="qch",
                                     name=f"qch0_{c}")
                    eng = nc.sync if c % 2 == 0 else nc.scalar
                    eng.dma_start(a[:], qTt[0, :, c * 2048:(c + 1) * 2048])
                    qch0.append(a)
                wk_sb = wpool.tile([P, ND * DK], bf16, tag="wk")
                nc.scalar.dma_start(wk_sb[:], wkt[:])
                wv_sb = wpool.tile([P, ND * DK], bf16, tag="wv")
                nc.sync.dma_start(wv_sb[:], wvt[:])
                vbuf = vbufp.tile([P, ND * T], bf16, tag="vbuf")

                deferred = []

                def flush(upto=None):
                    n = len(deferred) if upto is None else upto
                    for _ in range(n):
                        deferred.pop(0)()

                def norm_chain(xs_full, bias):
                    """Queue the scale-application for columns of
                    xs_full [128, 512] by e^bias/||col||.  Emits the
                    rowsum + ACT chain now; the PE broadcast + DVE
                    multiply are deferred so later matmuls can cover
                    the ACT latency."""
                    sq = sqpool.tile([P, 512], bf16, tag="sq")
                    nc.vector.tensor_mul(sq[:], xs_full, xs_full)
                    pm = psM.tile([P, 512], f32, tag="pm")
                    nc.tensor.matmul(pm[0:1, :], ones_col[:], sq[:],
                                     start=True, stop=True)
                    u = rows.tile([1, 512], f32, tag="u")
                    nc.scalar.activation(u[:], pm[0:1, :], AF.Ln)
                    iv = rows.tile([1, 512], bf16, tag="iv")
                    nc.scalar.activation(iv[:], u[:], AF.Exp,
                                         bias=bias, scale=-0.5)

                    def apply():
                        pb = psM.tile([P, 512], f32, tag="pm")
                        nc.tensor.matmul(pb[:], ones_row[:], iv[:],
                                         start=True, stop=True)
                        nc.vector.tensor_mul(xs_full, xs_full, pb[:])
                    deferred.append(apply)

                # K accumulators live across the whole interleaved loop.
                kaccs = [psA.tile([P, 512], f32, tag="acc",
                                  name=f"kacc{_t}") for _t in range(TB)]

                # Q-proj tb-major, K-proj quarters interleaved.
                for tb in range(TB):
                    if tb == 0:
                        qch = qch0
                    else:
                        qch = []
                        for c in range(4):
                            a = qstream.tile([P, 2048], bf16, tag="qch")
                            eng = nc.sync if c % 2 == 0 else nc.scalar
                            eng.dma_start(
                                a[:], qTt[tb, :, c * 2048:(c + 1) * 2048])
                            qch.append(a)
                        if tb == 2:
                            qgate = qch[0]
                            # WAR-gate the vT prefetch behind tb2's q
                            # chunk so its dispatches cannot be hoisted
                            # into the startup window; by now the Q/K
                            # streams are nearly drained.
                            for n in range(ND):
                                nc.vector.tensor_copy(
                                    vbuf[0:1, n * T:n * T + 8],
                                    qgate[0:1, 0:8])
                                eng = nc.sync if n % 2 == 0 else nc.scalar
                                eng.dma_start(
                                    vbuf[:, n * T:(n + 1) * T],
                                    vT[n * P:(n + 1) * P, :])
                    for h in range(HPG):
                        ps = psA.tile([P, 512], f32, tag="acc")
                        for n in range(ND):
                            nc.tensor.matmul(
                                ps[:],
                                wq_sb[:, n * E + h * P:n * E + (h + 1) * P],
                                qch[n // 4][:, (n % 4) * 512:
                                            (n % 4 + 1) * 512],
                                start=(n == 0), stop=(n == ND - 1))
                        flush()
                        xs = qt_sb[:, h * T + tb * 512:h * T + (tb + 1) * 512]
                        nc.vector.tensor_copy(xs, ps[:])
                        norm_chain(xs, lng_sb[0:1, h:h + 1])
                    # K quarter: 4 kT slices stream in, 16 matmuls.
                    for n in range(4 * tb, 4 * tb + 4):
                        a = kroll.tile([P, T], bf16, tag="kch")
                        nc.gpsimd.dma_start(a[:], kT[n * P:(n + 1) * P, :])
                        for t2 in range(TB):
                            nc.tensor.matmul(
                                kaccs[t2][:],
                                wk_sb[:, n * DK:(n + 1) * DK],
                                a[:, t2 * 512:(t2 + 1) * 512],
                                start=(n == 0), stop=(n == ND - 1))
                flush()

                # K copies + norm chains; V-proj provides PE cover.
                for t2 in range(TB):
                    xs = kt_sb[:, t2 * 512:(t2 + 1) * 512]
                    nc.vector.tensor_copy(xs, kaccs[t2][:])
                    norm_chain(xs, 0.0)

                vaccs = [psA.tile([P, 512], f32, tag="acc",
                                  name=f"vacc{_t}") for _t in range(TB)]
                for n in range(ND):
                    for t2 in range(TB):
                        nc.tensor.matmul(
                            vaccs[t2][:],
                            wv_sb[:, n * DK:(n + 1) * DK],
                            vbuf[:, n * T + t2 * 512:n * T + (t2 + 1) * 512],
                            start=(n == 0), stop=(n == ND - 1))
                    if n == 5:
                        flush()  # K bc-MMs + muls under V-proj cover
                vt_stage = vstage.tile([P, T], bf16, tag="vst")
                for t2 in range(TB):
                    nc.any.tensor_copy(
                        vt_stage[:, t2 * 512:(t2 + 1) * 512], vaccs[t2][:])
                # per-128-block transposes V^T -> V via the DMA XBAR
                # (SBUF->SBUF, off the PE entirely)
                for n in range(NT):
                    nc.sync.dma_start(
                        vtm_sb[:, n * P:(n + 1) * P],
                        vt_stage[:, n * P:(n + 1) * P], transpose=True)

            # ------------- phase B+C: attention + out projection ----------
            atp = outer.enter_context(tc.tile_pool(name="atp", bufs=3))
            rows2 = outer.enter_context(tc.tile_pool(name="rows2", bufs=4))
            wo_pool = outer.enter_context(tc.tile_pool(name="wo", bufs=1))
            ostage = outer.enter_context(tc.tile_pool(name="ostage", bufs=3))
            ps_st = outer.enter_context(
                tc.tile_pool(name="ps_st", bufs=2, space="PSUM"))
            ps_y = outer.enter_context(
                tc.tile_pool(name="ps_y", bufs=1, space="PSUM"))
            ps_misc = outer.enter_context(
                tc.tile_pool(name="ps_misc", bufs=1, space="PSUM"))
            ps_o = outer.enter_context(
                tc.tile_pool(name="ps_o", bufs=2, space="PSUM"))

            wo_sb = wo_pool.tile([P, HPG * D], bf16, tag="wo")
            # gate wo behind kt_sb so its 2MB cannot crowd the startup
            # streams; it lands well before the first out-projection.
            nc.vector.tensor_copy(wo_sb[0:1, 0:8], kt_sb[0:1, 0:8])
            nc.gpsimd.dma_start(wo_sb[:], wot[:])

            deferredB = []

            def flushB():
                while deferredB:
                    deferredB.pop(0)()

            for qb in range(TB):
                n_k = 4 * (qb + 1)
                # diagonal k-tiles first so the head's last exp has no
                # gpsimd select behind it; pairs stay j-aligned.
                ktiles = list(range(4 * qb, 4 * qb + 4)) + list(range(4 * qb))
                pairs = [ktiles[2 * i:2 * i + 2] for i in range(n_k // 2)]
                for h in range(HPG):
                    qh = qt_sb[:, h * T + qb * 512:h * T + (qb + 1) * 512]
                    strip = atp.tile([P, NT * 512], bf16, tag="strip")
                    # pm/ps_yt are allocated lazily at first use: their
                    # 1-buf pools are shared with the previous head's
                    # deferred broadcast tile, so allocation order must
                    # match PE emission order.
                    state = {}

                    def rowsum_y(pi, first, last, state=state):
                        if "pm" not in state:
                            state["pm"] = ps_misc.tile(
                                [P, 512], f32, tag="pm", name="pm")
                            state["y"] = ps_y.tile(
                                [P, 512], f32, tag="y", name="psyt")
                        pm, ps_yt = state["pm"], state["y"]
                        for kt in pairs[pi]:
                            j = kt - 4 * qb
                            off = 128 * j if j > 0 else 0
                            sl = strip[:, kt * 512 + off:(kt + 1) * 512]
                            nc.tensor.matmul(
                                pm[0:1, off:512], ones_col[:], sl,
                                start=(first and kt == pairs[pi][0]),
                                stop=(last and kt == pairs[pi][1]))
                        for kt in pairs[pi]:
                            j = kt - 4 * qb
                            off = 128 * j if j > 0 else 0
                            sl = strip[:, kt * 512 + off:(kt + 1) * 512]
                            nc.tensor.matmul(
                                ps_yt[:, off:512],
                                vtm_sb[:, kt * P:(kt + 1) * P], sl,
                                start=(first and kt == pairs[pi][0]),
                                stop=(last and kt == pairs[pi][1]))

                    for pi, pr in enumerate(pairs):
                        st = ps_st.tile([P, 1024], f32, tag="st")
                        if pi == 0:
                            flushB()  # prev head's bc-MM + scale
                        for j2, kt in enumerate(pr):
                            j = kt - 4 * qb
                            off = 128 * j if j > 0 else 0
                            nc.tensor.matmul(
                                st[:, j2 * 512 + off:(j2 + 1) * 512],
                                kt_sb[:, kt * P:(kt + 1) * P],
                                qh[:, off:512], start=True, stop=True)
                        ssl = strip[:, pr[0] * 512:pr[0] * 512 + 1024]
                        nc.scalar.activation(ssl, st[:], AF.Exp)
                        j0 = pr[0] - 4 * qb
                        if j0 >= 0:  # diagonal pair: causal zero-fill
                            nc.gpsimd.affine_select(
                                out=ssl, in_=ssl,
                                compare_op=mybir.AluOpType.is_ge,
                                fill=0.0, base=-128 * j0,
                                pattern=[[-128, 2], [1, 512]],
                                channel_multiplier=-1,
                            )
                        if pi > 0:
                            rowsum_y(pi - 1, first=(pi == 1), last=False)
                    rowsum_y(len(pairs) - 1, first=(len(pairs) == 1),
                             last=True)
                    pm, ps_yt = state["pm"], state["y"]
                    u = rows2.tile([1, 512], f32, tag="u")
                    nc.scalar.activation(u[:], pm[0:1, :], AF.Ln)
                    iv = rows2.tile([1, 512], bf16, tag="iv")
                    nc.scalar.activation(iv[:], u[:], AF.Exp, scale=-1.0)
                    yslice = yt_sb[:, h * T + qb * 512:h * T + (qb + 1) * 512]
                    nc.vector.tensor_copy(yslice, ps_yt[:])

                    def scale_y(iv=iv, yslice=yslice):
                        pb = ps_misc.tile([P, 512], f32, tag="pm")
                        nc.tensor.matmul(pb[:], ones_row[:], iv[:],
                                         start=True, stop=True)
                        nc.vector.tensor_mul(yslice, yslice, pb[:])
                    deferredB.append(scale_y)
                flushB()

                # out projection for this t-block (overlaps next q-block)
                tb = qb
                for ot in range(NT):
                    ps = ps_o.tile([P, 512], f32, tag="o")
                    for h in range(HPG):
                        nc.tensor.matmul(
                            ps[:],
                            wo_sb[:, h * D + ot * P:h * D + (ot + 1) * P],
                            yt_sb[:, h * T + tb * 512:h * T + (tb + 1) * 512],
                            start=(h == 0), stop=(h == HPG - 1))
                    o_sb = ostage.tile([P, 512], f32, tag="osb")
                    nc.any.tensor_copy(o_sb[:], ps[:])
                    nc.sync.dma_start(
                        outT[ot * P:(ot + 1) * P, tb * 512:(tb + 1) * 512],
                        o_sb[:])

    _single_act_table(nc.compile)
    return nc


def make_in_maps(q, k, v, Wq, Wk, Wv, Wo, g):
    import ml_dtypes
    st = ml_dtypes.bfloat16
    in_maps = []
    act_t = {}
    for b in range(B):
        qTb = np.ascontiguousarray(q[b].T).astype(st)
        # [TB, P, ND*512]: row p of block tb = concat_n qT[n*128+p, tb*512:]
        qTt = np.ascontiguousarray(
            qTb.reshape(ND, P, TB, 512).transpose(2, 1, 0, 3)
            .reshape(TB, P, ND * 512))
        act_t[b] = (
            qTt,
            np.ascontiguousarray(k[b].T).astype(st),
            np.ascontiguousarray(v[b].T).astype(st),
        )

    def wtile(wT, cols):  # wT: (D, cols) -> [P, ND*cols] row-tiled
        return np.ascontiguousarray(
            np.ascontiguousarray(wT).reshape(-1, P, cols)
            .transpose(1, 0, 2).reshape(P, -1)).astype(st)

    g_flat = np.asarray(g, dtype=np.float32).reshape(H)
    for c in range(8):
        b, gi = divmod(c, KVH)
        qTt, kTb, vTb = act_t[b]
        e0 = gi * E
        gvals = g_flat[gi * HPG:(gi + 1) * HPG] / math.sqrt(DK)
        in_maps.append({
            "qTt": qTt, "kT": kTb, "vT": vTb,
            "wqt": wtile(Wq[e0:e0 + E, :].T, E),
            "wkt": wtile(Wk[gi * DK:(gi + 1) * DK, :].T, DK),
            "wvt": wtile(Wv[gi * DK:(gi + 1) * DK, :].T, DK),
            "wot": wtile(Wo[:, e0:e0 + E].T, D),
            "lng": np.log(gvals)[None, :].astype(np.float32),
        })
    return in_maps


_cached = {}


def kernel(q, k, v, Wq, Wk, Wv, Wo, g, _trace=False, _tmpdir=None):
    if "nc" not in _cached:
        _cached["nc"] = build_kernel()
    nc = _cached["nc"]
    in_maps = make_in_maps(
        np.asarray(q, np.float32), np.asarray(k, np.float32),
        np.asarray(v, np.float32), np.asarray(Wq, np.float32),
        np.asarray(Wk, np.float32), np.asarray(Wv, np.float32),
        np.asarray(Wo, np.float32), g)
    res = run_bass_kernel_spmd(
        nc, in_maps, list(range(8)), trace=_trace, tmpdir=_tmpdir)
    out = np.empty((B, T, D), dtype=np.float32)
    for b in range(B):
        acc = res.results[4 * b]["outT"].copy()
        for gi in range(1, KVH):
            acc += res.results[4 * b + gi]["outT"]
        out[b] = acc.T
    kernel.last_results = res
    return out


# revision 19
# speedup vs baseline: 1.1577x; 1.0518x over previous
"""GQA attention with QK-norm for Trainium2, sharded over 8 NeuronCores.

Problem: B=2, T=2048, D=2048, H=16 query heads, KVH=4 kv heads, dk=128.
    Q = q @ Wq.T ; K = k @ Wk.T ; V = v @ Wv.T  (per batch)
    Q = g * l2norm(Q, per head) ; K = l2norm(K, per head)
    out = softmax(causal(Q K^T / sqrt(dk))) V @ Wo.T

Sharding: core c = 4*b + gi handles batch b and kv-head group gi
(4 query heads + 1 kv head). Each core computes a row-shard of the
output projection; the host sums the 4 partials per batch.

Structure (v6):
- DMA: the Q stream (wq + q-activation chunks) and the kT stream ride
  the two hardware DGE queues (sync/scalar, interleaved in first-need
  order); vT is a gpsimd-SWDGE prefetch WAR-gated behind tb1's first
  q chunk so it cannot crowd the startup window; wo is gated on
  kt_sb; outT stores alternate both HW queues.
- Q-proj runs tb-major interleaved with K-proj quarters so both
  streams are consumed at arrival pace; norm chains run inline per
  (head, tb): PE rowsum -> ACT Ln -> ACT Exp (gain folded in bias) ->
  deferred K=1 PE broadcast -> DVE multiply, hidden behind later
  matmuls.  All scalar-engine math lives in ONE activation table set
  (natural_log_exp_and_others); softmax inv = Exp(-Ln(rowsum)).
- V-proj reads the prefetched vbuf; V^T->V 128-blocks transpose via
  the DMA XBAR (no PE); the transposes overlap attention qb0 stage-1,
  whose Y matmuls run right after.
- Attention: per (h, q-block), k-tiles diagonal-first; rowsum/Y
  matmuls of pair p-1 interleave between S-matmul pairs p so the PE
  never waits on exp; causal masking is a gpsimd affine_select
  zero-fill; diagonal k-tiles only stream live q>=p+128j columns.
"""

import math
import sys

for _p in ("/opt/trn_rl_repo",):
    if _p not in sys.path:
        sys.path.append(_p)

import numpy as np
from concourse import bacc, mybir, tile
from concourse.bass_utils import run_bass_kernel_spmd

B, T, D, H, KVH, DK = 2, 2048, 2048, 16, 4, 128
HPG = H // KVH          # query heads per core (group)
E = HPG * DK            # 512: q-head dims per core
P = 128
TB = 4                  # t blocks of 512
NT = T // P             # 16 tiles of 128 along T
ND = D // P             # 16 contraction tiles
f32 = mybir.dt.float32
bf16 = mybir.dt.bfloat16
AF = mybir.ActivationFunctionType


def _single_act_table(fn):
    """Run fn with the ACT-table chooser forced to resolve every
    activation into natural_log_exp_and_others (contains Ln, Exp,
    Copy, Identity, Square -- everything this kernel uses).  The
    default chooser puts Exp and Ln in different sets and reloads
    tables (~1.3us) on every alternation.  Set ids stay valid: the
    dict keeps its length and insertion order; only the other sets'
    contents are hidden while our compile runs."""
    import concourse.bacc as _bm
    keep = "natural_log_exp_and_others"
    orig = _bm.get_activation_tables

    def patched(arch):
        t = orig(arch)
        return {name: (set(funcs) if name == keep else set())
                for name, funcs in t.items()}

    _bm.get_activation_tables = patched
    try:
        return fn()
    finally:
        _bm.get_activation_tables = orig


def build_kernel():
    nc = bacc.Bacc(None, target_bir_lowering=False)

    qTt = nc.declare_dram_parameter("qTt", [TB, P, ND * 512], bf16,
                                    isOutput=False)
    kT = nc.declare_dram_parameter("kT", [D, T], bf16, isOutput=False)
    vT = nc.declare_dram_parameter("vT", [D, T], bf16, isOutput=False)
    wqt = nc.declare_dram_parameter("wqt", [P, ND * E], bf16, isOutput=False)
    wkt = nc.declare_dram_parameter("wkt", [P, ND * DK], bf16, isOutput=False)
    wvt = nc.declare_dram_parameter("wvt", [P, ND * DK], bf16, isOutput=False)
    wot = nc.declare_dram_parameter("wot", [P, HPG * D], bf16, isOutput=False)
    lng = nc.declare_dram_parameter("lng", [1, HPG], f32, isOutput=False)
    outT = nc.declare_dram_parameter("outT", [D, T], f32, isOutput=True)

    from contextlib import ExitStack

    with tile.TileContext(nc) as tc:
        with ExitStack() as outer:
            const = outer.enter_context(tc.tile_pool(name="const", bufs=1))
            persist = outer.enter_context(tc.tile_pool(name="persist", bufs=1))

            ones_f32 = const.tile([P, 1], f32, tag="ones_f32")
            nc.vector.memset(ones_f32[:], 1.0)
            ones_col = const.tile([P, 1], bf16, tag="ones_col")
            nc.vector.tensor_copy(ones_col[:], ones_f32[:])
            onesr_f32 = const.tile([1, P], f32, tag="onesr_f32")
            nc.vector.memset(onesr_f32[:], 1.0)
            ones_row = const.tile([1, P], bf16, tag="ones_row")
            nc.vector.tensor_copy(ones_row[:], onesr_f32[:])
            lng_sb = const.tile([1, HPG], f32, tag="lng")
            nc.sync.dma_start(lng_sb[:], lng[:])

            qt_sb = persist.tile([P, HPG * T], bf16, tag="qt")
            kt_sb = persist.tile([P, T], bf16, tag="kt")
            vtm_sb = persist.tile([P, T], bf16, tag="vtm")
            yt_sb = persist.tile([P, HPG * T], bf16, tag="yt")

            q0state = []

            # ---------------- phase A: projections + norms ----------------
            with ExitStack() as pa:
                wpool = pa.enter_context(tc.tile_pool(name="wpool", bufs=1))
                vbufp = pa.enter_context(tc.tile_pool(name="vbuf", bufs=1))
                qstream = pa.enter_context(tc.tile_pool(name="qstream",
                                                        bufs=6))
                kroll = pa.enter_context(tc.tile_pool(name="kroll", bufs=6))
                vstage = pa.enter_context(tc.tile_pool(name="vstage", bufs=1))
                sqpool = pa.enter_context(tc.tile_pool(name="sqpool", bufs=2))
                rows = pa.enter_context(tc.tile_pool(name="rows", bufs=4))
                psA = pa.enter_context(
                    tc.tile_pool(name="psA", bufs=6, space="PSUM"))
                psM = pa.enter_context(
                    tc.tile_pool(name="psM", bufs=2, space="PSUM"))

                wq_sb = wpool.tile([P, ND * E], bf16, tag="wq")
                qch0 = []
                for c in range(4):
                    nc.sync.dma_start(
                        wq_sb[:, c * 2048:(c + 1) * 2048],
                        wqt[:, c * 2048:(c + 1) * 2048])
                    a = qstream.tile([P, 2048], bf16, tag="qch",
                                     name=f"qch0_{c}")
                    eng = nc.sync if c % 2 == 0 else nc.scalar
                    eng.dma_start(a[:], qTt[0, :, c * 2048:(c + 1) * 2048])
                    qch0.append(a)
                wk_sb = wpool.tile([P, ND * DK], bf16, tag="wk")
                nc.scalar.dma_start(wk_sb[:], wkt[:])
                wv_sb = wpool.tile([P, ND * DK], bf16, tag="wv")
                nc.sync.dma_start(wv_sb[:], wvt[:])
                vbuf = vbufp.tile([P, ND * T], bf16, tag="vbuf")

                deferred = []

                def flush(upto=None):
                    n = len(deferred) if upto is None else upto
                    for _ in range(n):
                        deferred.pop(0)()

                def norm_chain(xs_full, bias):
                    """Queue the scale-application for columns of
                    xs_full [128, 512] by e^bias/||col||.  Emits the
                    rowsum + ACT chain now; the PE broadcast + DVE
                    multiply are deferred so later matmuls can cover
                    the ACT latency."""
                    sq = sqpool.tile([P, 512], bf16, tag="sq")
                    nc.vector.tensor_mul(sq[:], xs_full, xs_full)
                    pm = psM.tile([P, 512], f32, tag="pm")
                    nc.tensor.matmul(pm[0:1, :], ones_col[:], sq[:],
                                     start=True, stop=True)
                    u = rows.tile([1, 512], f32, tag="u")
                    nc.scalar.activation(u[:], pm[0:1, :], AF.Ln)
                    iv = rows.tile([1, 512], bf16, tag="iv")
                    nc.scalar.activation(iv[:], u[:], AF.Exp,
                                         bias=bias, scale=-0.5)

                    def apply():
                        pb = psM.tile([P, 512], f32, tag="pm")
                        nc.tensor.matmul(pb[:], ones_row[:], iv[:],
                                         start=True, stop=True)
                        nc.vector.tensor_mul(xs_full, xs_full, pb[:])
                    deferred.append(apply)

                # K accumulators live across the whole interleaved loop.
                kaccs = [psA.tile([P, 512], f32, tag="acc",
                                  name=f"kacc{_t}") for _t in range(TB)]

                # Q-proj tb-major, K-proj quarters interleaved.
                for tb in range(TB):
                    if tb == 0:
                        qch = qch0
                    else:
                        qch = []
                        for c in range(4):
                            a = qstream.tile([P, 2048], bf16, tag="qch")
                            eng = nc.sync if c % 2 == 0 else nc.scalar
                            eng.dma_start(
                                a[:], qTt[tb, :, c * 2048:(c + 1) * 2048])
                            qch.append(a)
                        if tb == 1:
                            qgate = qch[0]
                            # WAR-gate the vT prefetch (gpsimd SWDGE)
                            # behind tb1's first q chunk so its
                            # dispatches cannot be hoisted into the
                            # startup window.
                            for n in range(ND):
                                nc.vector.tensor_copy(
                                    vbuf[0:1, n * T:n * T + 8],
                                    qgate[0:1, 0:8])
                                nc.gpsimd.dma_start(
                                    vbuf[:, n * T:(n + 1) * T],
                                    vT[n * P:(n + 1) * P, :])
                    for h in range(HPG):
                        ps = psA.tile([P, 512], f32, tag="acc")
                        for n in range(ND):
                            nc.tensor.matmul(
                                ps[:],
                                wq_sb[:, n * E + h * P:n * E + (h + 1) * P],
                                qch[n // 4][:, (n % 4) * 512:
                                            (n % 4 + 1) * 512],
                                start=(n == 0), stop=(n == ND - 1))
                        flush()
                        xs = qt_sb[:, h * T + tb * 512:h * T + (tb + 1) * 512]
                        nc.vector.tensor_copy(xs, ps[:])
                        norm_chain(xs, lng_sb[0:1, h:h + 1])
                    # K quarter: 4 kT slices stream in, 16 matmuls.
                    for n in range(4 * tb, 4 * tb + 4):
                        a = kroll.tile([P, T], bf16, tag="kch")
                        eng = nc.sync if n % 2 == 0 else nc.scalar
                        eng.dma_start(a[:], kT[n * P:(n + 1) * P, :])
                        for t2 in range(TB):
                            nc.tensor.matmul(
                                kaccs[t2][:],
                                wk_sb[:, n * DK:(n + 1) * DK],
                                a[:, t2 * 512:(t2 + 1) * 512],
                                start=(n == 0), stop=(n == ND - 1))
                flush()

                # K copies + norm chains; V-proj provides PE cover.
                for t2 in range(TB):
                    xs = kt_sb[:, t2 * 512:(t2 + 1) * 512]
                    nc.vector.tensor_copy(xs, kaccs[t2][:])
                    norm_chain(xs, 0.0)

                vaccs = [psA.tile([P, 512], f32, tag="acc",
                                  name=f"vacc{_t}") for _t in range(TB)]
                for n in range(ND):
                    for t2 in range(TB):
                        nc.tensor.matmul(
                            vaccs[t2][:],
                            wv_sb[:, n * DK:(n + 1) * DK],
                            vbuf[:, n * T + t2 * 512:n * T + (t2 + 1) * 512],
                            start=(n == 0), stop=(n == ND - 1))
                    if n == 5:
                        flush()  # K bc-MMs + muls under V-proj cover
                vt_stage = vstage.tile([P, T], bf16, tag="vst")
                for t2 in range(TB):
                    nc.any.tensor_copy(
                        vt_stage[:, t2 * 512:(t2 + 1) * 512], vaccs[t2][:])
                # per-128-block transposes V^T -> V via the DMA XBAR
                # (SBUF->SBUF, off the PE entirely); they overlap the
                # qb0 stage-1 matmuls emitted right after phase A.
                for n in range(NT):
                    eng = nc.sync if n % 2 == 0 else nc.scalar
                    eng.dma_start(
                        vtm_sb[:, n * P:(n + 1) * P],
                        vt_stage[:, n * P:(n + 1) * P], transpose=True)

            # ------------- phase B+C: attention + out projection ----------
            atp = outer.enter_context(tc.tile_pool(name="atp", bufs=3))
            q0strips = outer.enter_context(
                tc.tile_pool(name="q0strips", bufs=4))
            rows2 = outer.enter_context(tc.tile_pool(name="rows2", bufs=4))
            wo_pool = outer.enter_context(tc.tile_pool(name="wo", bufs=1))
            ostage = outer.enter_context(tc.tile_pool(name="ostage", bufs=3))
            ps_st = outer.enter_context(
                tc.tile_pool(name="ps_st", bufs=2, space="PSUM"))
            ps_yo = outer.enter_context(
                tc.tile_pool(name="ps_yo", bufs=2, space="PSUM"))
            psm2 = outer.enter_context(
                tc.tile_pool(name="psm2", bufs=2, space="PSUM"))

            wo_sb = wo_pool.tile([P, HPG * D], bf16, tag="wo")
            # gate wo behind kt_sb so its 2MB cannot crowd the startup
            # streams; it lands well before the first out-projection.
            nc.vector.tensor_copy(wo_sb[0:1, 0:8], kt_sb[0:1, 0:8])
            nc.gpsimd.dma_start(wo_sb[:], wot[:])

            # ---- qb0 stage-1: S/exp/mask/rowsum/inv (no V needed) ----
            for h in range(HPG):
                qh = qt_sb[:, h * T:h * T + 512]
                strip0 = q0strips.tile([P, 2048], bf16, tag="q0s")
                for pr in ([0, 1], [2, 3]):
                    st = ps_st.tile([P, 1024], f32, tag="st")
                    for j2, kt in enumerate(pr):
                        off = 128 * kt
                        nc.tensor.matmul(
                            st[:, j2 * 512 + off:(j2 + 1) * 512],
                            kt_sb[:, kt * P:(kt + 1) * P],
                            qh[:, off:512], start=True, stop=True)
                    ssl = strip0[:, pr[0] * 512:pr[0] * 512 + 1024]
                    nc.scalar.activation(ssl, st[:], AF.Exp)
                    nc.gpsimd.affine_select(
                        out=ssl, in_=ssl,
                        compare_op=mybir.AluOpType.is_ge,
                        fill=0.0, base=-128 * pr[0],
                        pattern=[[-128, 2], [1, 512]],
                        channel_multiplier=-1,
                    )
                pm = psm2.tile([P, 512], f32, tag="pm", name="pm0")
                for kt in range(4):
                    off = 128 * kt
                    nc.tensor.matmul(
                        pm[0:1, off:512], ones_col[:],
                        strip0[:, kt * 512 + off:(kt + 1) * 512],
                        start=(kt == 0), stop=(kt == 3))
                u = rows2.tile([1, 512], f32, tag="u")
                nc.scalar.activation(u[:], pm[0:1, :], AF.Ln)
                iv = rows2.tile([1, 512], bf16, tag="iv")
                nc.scalar.activation(iv[:], u[:], AF.Exp, scale=-1.0)
                q0state.append((strip0, iv))

            # ---- qb0 stage-2: Y + scaling + out-projection ----
            for h in range(HPG):
                strip0, iv = q0state[h]
                ps_yt = ps_yo.tile([P, 512], f32, tag="yo", name="y0")
                for kt in range(4):
                    off = 128 * kt
                    nc.tensor.matmul(
                        ps_yt[:, off:512], vtm_sb[:, kt * P:(kt + 1) * P],
                        strip0[:, kt * 512 + off:(kt + 1) * 512],
                        start=(kt == 0), stop=(kt == 3))
                yslice = yt_sb[:, h * T:h * T + 512]
                nc.vector.tensor_copy(yslice, ps_yt[:])
                pb = psm2.tile([P, 512], f32, tag="pm", name="pb0")
                nc.tensor.matmul(pb[:], ones_row[:], iv[:],
                                 start=True, stop=True)
                nc.vector.tensor_mul(yslice, yslice, pb[:])
            for ot in range(NT):
                ps = ps_yo.tile([P, 512], f32, tag="yo", name="o0")
                for h in range(HPG):
                    nc.tensor.matmul(
                        ps[:],
                        wo_sb[:, h * D + ot * P:h * D + (ot + 1) * P],
                        yt_sb[:, h * T:h * T + 512],
                        start=(h == 0), stop=(h == HPG - 1))
                o_sb = ostage.tile([P, 512], f32, tag="osb")
                nc.any.tensor_copy(o_sb[:], ps[:])
                eng = nc.sync if ot % 2 == 0 else nc.scalar
                eng.dma_start(outT[ot * P:(ot + 1) * P, 0:512], o_sb[:])

            deferredB = []

            def flushB():
                while deferredB:
                    deferredB.pop(0)()

            for qb in range(1, TB):
                n_k = 4 * (qb + 1)
                # diagonal k-tiles first so the head's last exp has no
                # gpsimd select behind it; pairs stay j-aligned.
                ktiles = list(range(4 * qb, 4 * qb + 4)) + list(range(4 * qb))
                pairs = [ktiles[2 * i:2 * i + 2] for i in range(n_k // 2)]
                for h in range(HPG):
                    qh = qt_sb[:, h * T + qb * 512:h * T + (qb + 1) * 512]
                    strip = atp.tile([P, NT * 512], bf16, tag="strip")
                    # pm/ps_yt allocated lazily at first use so pool
                    # allocation order matches PE emission order.
                    state = {}

                    def rowsum_y(pi, first, last, state=state, pairs=pairs,
                                 qb=qb, strip=strip):
                        if "pm" not in state:
                            state["pm"] = psm2.tile(
                                [P, 512], f32, tag="pm", name="pm")
                            state["y"] = ps_yo.tile(
                                [P, 512], f32, tag="yo", name="psyt")
                        pm, ps_yt = state["pm"], state["y"]
                        for kt in pairs[pi]:
                            j = kt - 4 * qb
                            off = 128 * j if j > 0 else 0
                            sl = strip[:, kt * 512 + off:(kt + 1) * 512]
                            nc.tensor.matmul(
                                pm[0:1, off:512], ones_col[:], sl,
                                start=(first and kt == pairs[pi][0]),
                                stop=(last and kt == pairs[pi][1]))
                        for kt in pairs[pi]:
                            j = kt - 4 * qb
                            off = 128 * j if j > 0 else 0
                            sl = strip[:, kt * 512 + off:(kt + 1) * 512]
                            nc.tensor.matmul(
                                ps_yt[:, off:512],
                                vtm_sb[:, kt * P:(kt + 1) * P], sl,
                                start=(first and kt == pairs[pi][0]),
                                stop=(last and kt == pairs[pi][1]))

                    for pi, pr in enumerate(pairs):
                        st = ps_st.tile([P, 1024], f32, tag="st")
                        if pi == 0:
                            flushB()  # prev head's bc-MM + scale
                        for j2, kt in enumerate(pr):
                            j = kt - 4 * qb
                            off = 128 * j if j > 0 else 0
                            nc.tensor.matmul(
                                st[:, j2 * 512 + off:(j2 + 1) * 512],
                                kt_sb[:, kt * P:(kt + 1) * P],
                                qh[:, off:512], start=True, stop=True)
                        ssl = strip[:, pr[0] * 512:pr[0] * 512 + 1024]
                        nc.scalar.activation(ssl, st[:], AF.Exp)
                        j0 = pr[0] - 4 * qb
                        if j0 >= 0:  # diagonal pair: causal zero-fill
                            nc.gpsimd.affine_select(
                                out=ssl, in_=ssl,
                                compare_op=mybir.AluOpType.is_ge,
                                fill=0.0, base=-128 * j0,
                                pattern=[[-128, 2], [1, 512]],
                                channel_multiplier=-1,
                            )
                        if pi > 0:
                            rowsum_y(pi - 1, first=(pi == 1), last=False)
                    rowsum_y(len(pairs) - 1, first=(len(pairs) == 1),
                             last=True)
                    pm, ps_yt = state["pm"], state["y"]
                    u = rows2.tile([1, 512], f32, tag="u")
                    nc.scalar.activation(u[:], pm[0:1, :], AF.Ln)
                    iv = rows2.tile([1, 512], bf16, tag="iv")
                    nc.scalar.activation(iv[:], u[:], AF.Exp, scale=-1.0)
                    yslice = yt_sb[:, h * T + qb * 512:h * T + (qb + 1) * 512]
                    nc.vector.tensor_copy(yslice, ps_yt[:])

                    def scale_y(iv=iv, yslice=yslice):
                        pb = psm2.tile([P, 512], f32, tag="pm", name="pb")
                        nc.tensor.matmul(pb[:], ones_row[:], iv[:],
                                         start=True, stop=True)
                        nc.vector.tensor_mul(yslice, yslice, pb[:])
                    deferredB.append(scale_y)
                flushB()

                # out projection for this t-block (overlaps next q-block)
                tb = qb
                for ot in range(NT):
                    ps = ps_yo.tile([P, 512], f32, tag="yo", name="o")
                    for h in range(HPG):
                        nc.tensor.matmul(
                            ps[:],
                            wo_sb[:, h * D + ot * P:h * D + (ot + 1) * P],
                            yt_sb[:, h * T + tb * 512:h * T + (tb + 1) * 512],
                            start=(h == 0), stop=(h == HPG - 1))
                    o_sb = ostage.tile([P, 512], f32, tag="osb")
                    nc.any.tensor_copy(o_sb[:], ps[:])
                    eng = nc.sync if ot % 2 == 0 else nc.scalar
                    eng.dma_start(
                        outT[ot * P:(ot + 1) * P, tb * 512:(tb + 1) * 512],
                        o_sb[:])

    _single_act_table(nc.compile)
    return nc


def make_in_maps(q, k, v, Wq, Wk, Wv, Wo, g):
    import ml_dtypes
    st = ml_dtypes.bfloat16
    in_maps = []
    act_t = {}
    for b in range(B):
        qTb = np.ascontiguousarray(q[b].T).astype(st)
        # [TB, P, ND*512]: row p of block tb = concat_n qT[n*128+p, tb*512:]
        qTt = np.ascontiguousarray(
            qTb.reshape(ND, P, TB, 512).transpose(2, 1, 0, 3)
            .reshape(TB, P, ND * 512))
        act_t[b] = (
            qTt,
            np.ascontiguousarray(k[b].T).astype(st),
            np.ascontiguousarray(v[b].T).astype(st),
        )

    def wtile(wT, cols):  # wT: (D, cols) -> [P, ND*cols] row-tiled
        return np.ascontiguousarray(
            np.ascontiguousarray(wT).reshape(-1, P, cols)
            .transpose(1, 0, 2).reshape(P, -1)).astype(st)

    g_flat = np.asarray(g, dtype=np.float32).reshape(H)
    for c in range(8):
        b, gi = divmod(c, KVH)
        qTt, kTb, vTb = act_t[b]
        e0 = gi * E
        gvals = g_flat[gi * HPG:(gi + 1) * HPG] / math.sqrt(DK)
        in_maps.append({
            "qTt": qTt, "kT": kTb, "vT": vTb,
            "wqt": wtile(Wq[e0:e0 + E, :].T, E),
            "wkt": wtile(Wk[gi * DK:(gi + 1) * DK, :].T, DK),
            "wvt": wtile(Wv[gi * DK:(gi + 1) * DK, :].T, DK),
            "wot": wtile(Wo[:, e0:e0 + E].T, D),
            "lng": np.log(gvals)[None, :].astype(np.float32),
        })
    return in_maps


_cached = {}


def kernel(q, k, v, Wq, Wk, Wv, Wo, g, _trace=False, _tmpdir=None):
    if "nc" not in _cached:
        _cached["nc"] = build_kernel()
    nc = _cached["nc"]
    in_maps = make_in_maps(
        np.asarray(q, np.float32), np.asarray(k, np.float32),
        np.asarray(v, np.float32), np.asarray(Wq, np.float32),
        np.asarray(Wk, np.float32), np.asarray(Wv, np.float32),
        np.asarray(Wo, np.float32), g)
    res = run_bass_kernel_spmd(
        nc, in_maps, list(range(8)), trace=_trace, tmpdir=_tmpdir)
    out = np.empty((B, T, D), dtype=np.float32)
    for b in range(B):
        acc = res.results[4 * b]["outT"].copy()
        for gi in range(1, KVH):
            acc += res.results[4 * b + gi]["outT"]
        out[b] = acc.T
    kernel.last_results = res
    return out


# revision 20
# speedup vs baseline: 1.2350x; 1.0668x over previous
"""GQA attention with QK-norm for Trainium2, sharded over 8 NeuronCores.

Problem: B=2, T=2048, D=2048, H=16 query heads, KVH=4 kv heads, dk=128.
    Q = q @ Wq.T ; K = k @ Wk.T ; V = v @ Wv.T  (per batch)
    Q = g * l2norm(Q, per head) ; K = l2norm(K, per head)
    out = softmax(causal(Q K^T / sqrt(dk))) V @ Wo.T

Sharding: core c = 4*b + gi handles batch b and kv-head group gi
(4 query heads + 1 kv head). Each core computes a row-shard of the
output projection; the host sums the 4 partials per batch.

Structure (v6):
- DMA: the Q stream (wq + q-activation chunks) and the kT stream ride
  the two hardware DGE queues (sync/scalar, interleaved in first-need
  order); vT is a gpsimd-SWDGE prefetch WAR-gated behind tb1's first
  q chunk so it cannot crowd the startup window; wo is gated on
  kt_sb; outT stores alternate both HW queues.
- Q-proj runs tb-major interleaved with K-proj quarters so both
  streams are consumed at arrival pace; norm chains run inline per
  (head, tb): PE rowsum -> ACT Ln -> ACT Exp (gain folded in bias) ->
  deferred K=1 PE broadcast -> DVE multiply, hidden behind later
  matmuls.  All scalar-engine math lives in ONE activation table set
  (natural_log_exp_and_others); softmax inv = Exp(-Ln(rowsum)).
- V-proj reads the prefetched vbuf; V^T->V 128-blocks transpose via
  the DMA XBAR (no PE); the transposes overlap attention qb0 stage-1,
  whose Y matmuls run right after.
- Attention: per (h, q-block), k-tiles diagonal-first; rowsum/Y
  matmuls of pair p-1 interleave between S-matmul pairs p so the PE
  never waits on exp; causal masking is a gpsimd affine_select
  zero-fill; diagonal k-tiles only stream live q>=p+128j columns.
"""

import math
import sys

for _p in ("/opt/trn_rl_repo",):
    if _p not in sys.path:
        sys.path.append(_p)

import numpy as np
from concourse import bacc, mybir, tile
from concourse.bass_utils import run_bass_kernel_spmd
from concourse.masks import make_identity

B, T, D, H, KVH, DK = 2, 2048, 2048, 16, 4, 128
HPG = H // KVH          # query heads per core (group)
E = HPG * DK            # 512: q-head dims per core
P = 128
TB = 4                  # t blocks of 512
NT = T // P             # 16 tiles of 128 along T
ND = D // P             # 16 contraction tiles
f32 = mybir.dt.float32
bf16 = mybir.dt.bfloat16
AF = mybir.ActivationFunctionType


def _single_act_table(fn):
    """Run fn with the ACT-table chooser forced to resolve every
    activation into natural_log_exp_and_others (contains Ln, Exp,
    Copy, Identity, Square -- everything this kernel uses).  The
    default chooser puts Exp and Ln in different sets and reloads
    tables (~1.3us) on every alternation.  Set ids stay valid: the
    dict keeps its length and insertion order; only the other sets'
    contents are hidden while our compile runs."""
    import concourse.bacc as _bm
    keep = "natural_log_exp_and_others"
    orig = _bm.get_activation_tables

    def patched(arch):
        t = orig(arch)
        return {name: (set(funcs) if name == keep else set())
                for name, funcs in t.items()}

    _bm.get_activation_tables = patched
    try:
        return fn()
    finally:
        _bm.get_activation_tables = orig


def build_kernel():
    nc = bacc.Bacc(None, target_bir_lowering=False)

    qTt = nc.declare_dram_parameter("qTt", [TB, P, ND * 512], bf16,
                                    isOutput=False)
    kT = nc.declare_dram_parameter("kT", [D, T], bf16, isOutput=False)
    vT = nc.declare_dram_parameter("vT", [D, T], bf16, isOutput=False)
    wqt = nc.declare_dram_parameter("wqt", [P, ND * E], bf16, isOutput=False)
    wkt = nc.declare_dram_parameter("wkt", [P, ND * DK], bf16, isOutput=False)
    wvt = nc.declare_dram_parameter("wvt", [P, ND * DK], bf16, isOutput=False)
    wot = nc.declare_dram_parameter("wot", [P, HPG * D], bf16, isOutput=False)
    lng = nc.declare_dram_parameter("lng", [1, HPG], f32, isOutput=False)
    outT = nc.declare_dram_parameter("outT", [D, T], f32, isOutput=True)

    from contextlib import ExitStack

    with tile.TileContext(nc) as tc:
        with ExitStack() as outer:
            const = outer.enter_context(tc.tile_pool(name="const", bufs=1))
            persist = outer.enter_context(tc.tile_pool(name="persist", bufs=1))

            ident_f32 = const.tile([P, P], f32, tag="ident_f32")
            make_identity(nc, ident_f32[:])
            identb = const.tile([P, P], bf16, tag="identb")
            nc.vector.tensor_copy(identb[:], ident_f32[:])
            ones_f32 = const.tile([P, 1], f32, tag="ones_f32")
            nc.vector.memset(ones_f32[:], 1.0)
            ones_col = const.tile([P, 1], bf16, tag="ones_col")
            nc.vector.tensor_copy(ones_col[:], ones_f32[:])
            onesr_f32 = const.tile([1, P], f32, tag="onesr_f32")
            nc.vector.memset(onesr_f32[:], 1.0)
            ones_row = const.tile([1, P], bf16, tag="ones_row")
            nc.vector.tensor_copy(ones_row[:], onesr_f32[:])
            lng_sb = const.tile([1, HPG], f32, tag="lng")
            nc.sync.dma_start(lng_sb[:], lng[:])

            qt_sb = persist.tile([P, HPG * T], bf16, tag="qt")
            kt_sb = persist.tile([P, T], bf16, tag="kt")
            vtm_sb = persist.tile([P, T], bf16, tag="vtm")
            yt_sb = persist.tile([P, HPG * T], bf16, tag="yt")

            q0state = []

            # ---------------- phase A: projections + norms ----------------
            with ExitStack() as pa:
                wpool = pa.enter_context(tc.tile_pool(name="wpool", bufs=1))
                vbufp = pa.enter_context(tc.tile_pool(name="vbuf", bufs=1))
                qstream = pa.enter_context(tc.tile_pool(name="qstream",
                                                        bufs=6))
                kroll = pa.enter_context(tc.tile_pool(name="kroll", bufs=6))
                vstage = pa.enter_context(tc.tile_pool(name="vstage", bufs=1))
                sqpool = pa.enter_context(tc.tile_pool(name="sqpool", bufs=2))
                rows = pa.enter_context(tc.tile_pool(name="rows", bufs=4))
                psA = pa.enter_context(
                    tc.tile_pool(name="psA", bufs=6, space="PSUM"))
                psM = pa.enter_context(
                    tc.tile_pool(name="psM", bufs=2, space="PSUM"))

                wq_sb = wpool.tile([P, ND * E], bf16, tag="wq")
                qch0 = []
                for c in range(4):
                    nc.sync.dma_start(
                        wq_sb[:, c * 2048:(c + 1) * 2048],
                        wqt[:, c * 2048:(c + 1) * 2048])
                    a = qstream.tile([P, 2048], bf16, tag="qch",
                                     name=f"qch0_{c}")
                    eng = nc.sync if c % 2 == 0 else nc.scalar
                    eng.dma_start(a[:], qTt[0, :, c * 2048:(c + 1) * 2048])
                    qch0.append(a)
                wk_sb = wpool.tile([P, ND * DK], bf16, tag="wk")
                nc.scalar.dma_start(wk_sb[:], wkt[:])
                wv_sb = wpool.tile([P, ND * DK], bf16, tag="wv")
                nc.sync.dma_start(wv_sb[:], wvt[:])
                vbuf = vbufp.tile([P, ND * T], bf16, tag="vbuf")

                deferred = []

                def flush(upto=None):
                    n = len(deferred) if upto is None else upto
                    for _ in range(n):
                        deferred.pop(0)()

                def norm_chain(xs_full, bias):
                    """Queue the scale-application for columns of
                    xs_full [128, 512] by e^bias/||col||.  Emits the
                    rowsum + ACT chain now; the PE broadcast + DVE
                    multiply are deferred so later matmuls can cover
                    the ACT latency."""
                    sq = sqpool.tile([P, 512], bf16, tag="sq")
                    nc.vector.tensor_mul(sq[:], xs_full, xs_full)
                    pm = psM.tile([P, 512], f32, tag="pm")
                    nc.tensor.matmul(pm[0:1, :], ones_col[:], sq[:],
                                     start=True, stop=True)
                    u = rows.tile([1, 512], f32, tag="u")
                    nc.scalar.activation(u[:], pm[0:1, :], AF.Ln)
                    iv = rows.tile([1, 512], bf16, tag="iv")
                    nc.scalar.activation(iv[:], u[:], AF.Exp,
                                         bias=bias, scale=-0.5)

                    def apply():
                        pb = psM.tile([P, 512], f32, tag="pm")
                        nc.tensor.matmul(pb[:], ones_row[:], iv[:],
                                         start=True, stop=True)
                        nc.vector.tensor_mul(xs_full, xs_full, pb[:])
                    deferred.append(apply)

                # K accumulators live across the whole interleaved loop.
                kaccs = [psA.tile([P, 512], f32, tag="acc",
                                  name=f"kacc{_t}") for _t in range(TB)]

                # Q-proj tb-major, K-proj quarters interleaved.
                for tb in range(TB):
                    if tb == 0:
                        qch = qch0
                    else:
                        qch = []
                        for c in range(4):
                            a = qstream.tile([P, 2048], bf16, tag="qch")
                            eng = nc.sync if c % 2 == 0 else nc.scalar
                            eng.dma_start(
                                a[:], qTt[tb, :, c * 2048:(c + 1) * 2048])
                            qch.append(a)
                        if tb == 1:
                            qgate = qch[0]
                            # WAR-gate the vT prefetch (gpsimd SWDGE)
                            # behind tb1's first q chunk so its
                            # dispatches cannot be hoisted into the
                            # startup window.
                            for n in range(ND):
                                nc.vector.tensor_copy(
                                    vbuf[0:1, n * T:n * T + 8],
                                    qgate[0:1, 0:8])
                                nc.gpsimd.dma_start(
                                    vbuf[:, n * T:(n + 1) * T],
                                    vT[n * P:(n + 1) * P, :])
                    for h in range(HPG):
                        ps = psA.tile([P, 512], f32, tag="acc")
                        for n in range(ND):
                            nc.tensor.matmul(
                                ps[:],
                                wq_sb[:, n * E + h * P:n * E + (h + 1) * P],
                                qch[n // 4][:, (n % 4) * 512:
                                            (n % 4 + 1) * 512],
                                start=(n == 0), stop=(n == ND - 1))
                        flush()
                        xs = qt_sb[:, h * T + tb * 512:h * T + (tb + 1) * 512]
                        nc.vector.tensor_copy(xs, ps[:])
                        norm_chain(xs, lng_sb[0:1, h:h + 1])
                    # K quarter: 4 kT slices stream in, 16 matmuls.
                    for n in range(4 * tb, 4 * tb + 4):
                        a = kroll.tile([P, T], bf16, tag="kch")
                        eng = nc.sync if n % 2 == 0 else nc.scalar
                        eng.dma_start(a[:], kT[n * P:(n + 1) * P, :])
                        for t2 in range(TB):
                            nc.tensor.matmul(
                                kaccs[t2][:],
                                wk_sb[:, n * DK:(n + 1) * DK],
                                a[:, t2 * 512:(t2 + 1) * 512],
                                start=(n == 0), stop=(n == ND - 1))
                flush()

                # K copies + norm chains; V-proj provides PE cover.
                for t2 in range(TB):
                    xs = kt_sb[:, t2 * 512:(t2 + 1) * 512]
                    nc.vector.tensor_copy(xs, kaccs[t2][:])
                    norm_chain(xs, 0.0)

                vaccs = [psA.tile([P, 512], f32, tag="acc",
                                  name=f"vacc{_t}") for _t in range(TB)]
                for n in range(ND):
                    for t2 in range(TB):
                        nc.tensor.matmul(
                            vaccs[t2][:],
                            wv_sb[:, n * DK:(n + 1) * DK],
                            vbuf[:, n * T + t2 * 512:n * T + (t2 + 1) * 512],
                            start=(n == 0), stop=(n == ND - 1))
                    if n == 5:
                        flush()  # K bc-MMs + muls under V-proj cover
                vt_stage = vstage.tile([P, T], bf16, tag="vst")
                for t2 in range(TB):
                    nc.any.tensor_copy(
                        vt_stage[:, t2 * 512:(t2 + 1) * 512], vaccs[t2][:])
                # per-128-block transposes V^T -> V on the PE (psM
                # tiles are free after the K broadcasts).  DMA XBAR
                # transposes were tried and cost ~1.2us EACH of
                # hardware-DGE engine dispatch, starving the scalar
                # engine's exp dispatches.
                for n in range(NT):
                    tp = psM.tile([P, P], bf16, tag="pm", name="tp")
                    nc.tensor.transpose(
                        tp[:], vt_stage[:, n * P:(n + 1) * P], identb[:])
                    nc.any.tensor_copy(vtm_sb[:, n * P:(n + 1) * P], tp[:])

            # ------------- phase B+C: attention + out projection ----------
            atp = outer.enter_context(tc.tile_pool(name="atp", bufs=3))
            q0strips = outer.enter_context(
                tc.tile_pool(name="q0strips", bufs=4))
            rows2 = outer.enter_context(tc.tile_pool(name="rows2", bufs=4))
            wo_pool = outer.enter_context(tc.tile_pool(name="wo", bufs=1))
            ostage = outer.enter_context(tc.tile_pool(name="ostage", bufs=3))
            ps_st = outer.enter_context(
                tc.tile_pool(name="ps_st", bufs=2, space="PSUM"))
            ps_yo = outer.enter_context(
                tc.tile_pool(name="ps_yo", bufs=2, space="PSUM"))
            psm2 = outer.enter_context(
                tc.tile_pool(name="psm2", bufs=2, space="PSUM"))

            wo_sb = wo_pool.tile([P, HPG * D], bf16, tag="wo")
            # gate wo behind kt_sb so its 2MB cannot crowd the startup
            # streams; it lands well before the first out-projection.
            nc.vector.tensor_copy(wo_sb[0:1, 0:8], kt_sb[0:1, 0:8])
            nc.gpsimd.dma_start(wo_sb[:], wot[:])

            # ---- qb0 stage-1: S/exp/mask/rowsum/inv (no V needed) ----
            for h in range(HPG):
                qh = qt_sb[:, h * T:h * T + 512]
                strip0 = q0strips.tile([P, 2048], bf16, tag="q0s")
                for pr in ([0, 1], [2, 3]):
                    st = ps_st.tile([P, 1024], f32, tag="st")
                    for j2, kt in enumerate(pr):
                        off = 128 * kt
                        nc.tensor.matmul(
                            st[:, j2 * 512 + off:(j2 + 1) * 512],
                            kt_sb[:, kt * P:(kt + 1) * P],
                            qh[:, off:512], start=True, stop=True)
                    ssl = strip0[:, pr[0] * 512:pr[0] * 512 + 1024]
                    nc.scalar.activation(ssl, st[:], AF.Exp)
                    nc.gpsimd.affine_select(
                        out=ssl, in_=ssl,
                        compare_op=mybir.AluOpType.is_ge,
                        fill=0.0, base=-128 * pr[0],
                        pattern=[[-128, 2], [1, 512]],
                        channel_multiplier=-1,
                    )
                pm = psm2.tile([P, 512], f32, tag="pm", name="pm0")
                for kt in range(4):
                    off = 128 * kt
                    nc.tensor.matmul(
                        pm[0:1, off:512], ones_col[:],
                        strip0[:, kt * 512 + off:(kt + 1) * 512],
                        start=(kt == 0), stop=(kt == 3))
                u = rows2.tile([1, 512], f32, tag="u")
                nc.scalar.activation(u[:], pm[0:1, :], AF.Ln)
                iv = rows2.tile([1, 512], bf16, tag="iv")
                nc.scalar.activation(iv[:], u[:], AF.Exp, scale=-1.0)
                q0state.append((strip0, iv))

            # ---- qb0 stage-2: Y + scaling + out-projection ----
            for h in range(HPG):
                strip0, iv = q0state[h]
                ps_yt = ps_yo.tile([P, 512], f32, tag="yo", name="y0")
                for kt in range(4):
                    off = 128 * kt
                    nc.tensor.matmul(
                        ps_yt[:, off:512], vtm_sb[:, kt * P:(kt + 1) * P],
                        strip0[:, kt * 512 + off:(kt + 1) * 512],
                        start=(kt == 0), stop=(kt == 3))
                yslice = yt_sb[:, h * T:h * T + 512]
                nc.vector.tensor_copy(yslice, ps_yt[:])
                pb = psm2.tile([P, 512], f32, tag="pm", name="pb0")
                nc.tensor.matmul(pb[:], ones_row[:], iv[:],
                                 start=True, stop=True)
                nc.vector.tensor_mul(yslice, yslice, pb[:])
            for ot in range(NT):
                ps = ps_yo.tile([P, 512], f32, tag="yo", name="o0")
                for h in range(HPG):
                    nc.tensor.matmul(
                        ps[:],
                        wo_sb[:, h * D + ot * P:h * D + (ot + 1) * P],
                        yt_sb[:, h * T:h * T + 512],
                        start=(h == 0), stop=(h == HPG - 1))
                o_sb = ostage.tile([P, 512], f32, tag="osb")
                nc.any.tensor_copy(o_sb[:], ps[:])
                eng = nc.sync if ot % 2 == 0 else nc.scalar
                eng.dma_start(outT[ot * P:(ot + 1) * P, 0:512], o_sb[:])

            deferredB = []

            def flushB():
                while deferredB:
                    deferredB.pop(0)()

            for qb in range(1, TB):
                n_k = 4 * (qb + 1)
                # diagonal k-tiles first so the head's last exp has no
                # gpsimd select behind it; pairs stay j-aligned.
                ktiles = list(range(4 * qb, 4 * qb + 4)) + list(range(4 * qb))
                pairs = [ktiles[2 * i:2 * i + 2] for i in range(n_k // 2)]
                for h in range(HPG):
                    qh = qt_sb[:, h * T + qb * 512:h * T + (qb + 1) * 512]
                    strip = atp.tile([P, NT * 512], bf16, tag="strip")
                    # pm/ps_yt allocated lazily at first use so pool
                    # allocation order matches PE emission order.
                    state = {}

                    def rowsum_y(pi, first, last, state=state, pairs=pairs,
                                 qb=qb, strip=strip):
                        if "pm" not in state:
                            state["pm"] = psm2.tile(
                                [P, 512], f32, tag="pm", name="pm")
                            state["y"] = ps_yo.tile(
                                [P, 512], f32, tag="yo", name="psyt")
                        pm, ps_yt = state["pm"], state["y"]
                        for kt in pairs[pi]:
                            j = kt - 4 * qb
                            off = 128 * j if j > 0 else 0
                            sl = strip[:, kt * 512 + off:(kt + 1) * 512]
                            nc.tensor.matmul(
                                pm[0:1, off:512], ones_col[:], sl,
                                start=(first and kt == pairs[pi][0]),
                                stop=(last and kt == pairs[pi][1]))
                        for kt in pairs[pi]:
                            j = kt - 4 * qb
                            off = 128 * j if j > 0 else 0
                            sl = strip[:, kt * 512 + off:(kt + 1) * 512]
                            nc.tensor.matmul(
                                ps_yt[:, off:512],
                                vtm_sb[:, kt * P:(kt + 1) * P], sl,
                                start=(first and kt == pairs[pi][0]),
                                stop=(last and kt == pairs[pi][1]))

                    for pi, pr in enumerate(pairs):
                        st = ps_st.tile([P, 1024], f32, tag="st")
                        if pi == 0:
                            flushB()  # prev head's bc-MM + scale
                        for j2, kt in enumerate(pr):
                            j = kt - 4 * qb
                            off = 128 * j if j > 0 else 0
                            nc.tensor.matmul(
                                st[:, j2 * 512 + off:(j2 + 1) * 512],
                                kt_sb[:, kt * P:(kt + 1) * P],
                                qh[:, off:512], start=True, stop=True)
                        ssl = strip[:, pr[0] * 512:pr[0] * 512 + 1024]
                        nc.scalar.activation(ssl, st[:], AF.Exp)
                        j0 = pr[0] - 4 * qb
                        if j0 >= 0:  # diagonal pair: causal zero-fill
                            nc.gpsimd.affine_select(
                                out=ssl, in_=ssl,
                                compare_op=mybir.AluOpType.is_ge,
                                fill=0.0, base=-128 * j0,
                                pattern=[[-128, 2], [1, 512]],
                                channel_multiplier=-1,
                            )
                        if pi > 0:
                            rowsum_y(pi - 1, first=(pi == 1), last=False)
                    rowsum_y(len(pairs) - 1, first=(len(pairs) == 1),
                             last=True)
                    pm, ps_yt = state["pm"], state["y"]
                    u = rows2.tile([1, 512], f32, tag="u")
                    nc.scalar.activation(u[:], pm[0:1, :], AF.Ln)
                    iv = rows2.tile([1, 512], bf16, tag="iv")
                    nc.scalar.activation(iv[:], u[:], AF.Exp, scale=-1.0)
                    yslice = yt_sb[:, h * T + qb * 512:h * T + (qb + 1) * 512]
                    nc.vector.tensor_copy(yslice, ps_yt[:])

                    def scale_y(iv=iv, yslice=yslice):
                        pb = psm2.tile([P, 512], f32, tag="pm", name="pb")
                        nc.tensor.matmul(pb[:], ones_row[:], iv[:],
                                         start=True, stop=True)
                        nc.vector.tensor_mul(yslice, yslice, pb[:])
                    deferredB.append(scale_y)
                flushB()

                # out projection for this t-block (overlaps next q-block)
                tb = qb
                for ot in range(NT):
                    ps = ps_yo.tile([P, 512], f32, tag="yo", name="o")
                    for h in range(HPG):
                        nc.tensor.matmul(
                            ps[:],
                            wo_sb[:, h * D + ot * P:h * D + (ot + 1) * P],
                            yt_sb[:, h * T + tb * 512:h * T + (tb + 1) * 512],
                            start=(h == 0), stop=(h == HPG - 1))
                    o_sb = ostage.tile([P, 512], f32, tag="osb")
                    nc.any.tensor_copy(o_sb[:], ps[:])
                    eng = nc.sync if ot % 2 == 0 else nc.scalar
                    eng.dma_start(
                        outT[ot * P:(ot + 1) * P, tb * 512:(tb + 1) * 512],
                        o_sb[:])

    _single_act_table(nc.compile)
    return nc


def make_in_maps(q, k, v, Wq, Wk, Wv, Wo, g):
    import ml_dtypes
    st = ml_dtypes.bfloat16
    in_maps = []
    act_t = {}
    for b in range(B):
        qTb = np.ascontiguousarray(q[b].T).astype(st)
        # [TB, P, ND*512]: row p of block tb = concat_n qT[n*128+p, tb*512:]
        qTt = np.ascontiguousarray(
            qTb.reshape(ND, P, TB, 512).transpose(2, 1, 0, 3)
            .reshape(TB, P, ND * 512))
        act_t[b] = (
            qTt,
            np.ascontiguousarray(k[b].T).astype(st),
            np.ascontiguousarray(v[b].T).astype(st),
        )

    def wtile(wT, cols):  # wT: (D, cols) -> [P, ND*cols] row-tiled
        return np.ascontiguousarray(
            np.ascontiguousarray(wT).reshape(-1, P, cols)
            .transpose(1, 0, 2).reshape(P, -1)).astype(st)

    g_flat = np.asarray(g, dtype=np.float32).reshape(H)
    for c in range(8):
        b, gi = divmod(c, KVH)
        qTt, kTb, vTb = act_t[b]
        e0 = gi * E
        gvals = g_flat[gi * HPG:(gi + 1) * HPG] / math.sqrt(DK)
        in_maps.append({
            "qTt": qTt, "kT": kTb, "vT": vTb,
            "wqt": wtile(Wq[e0:e0 + E, :].T, E),
            "wkt": wtile(Wk[gi * DK:(gi + 1) * DK, :].T, DK),
            "wvt": wtile(Wv[gi * DK:(gi + 1) * DK, :].T, DK),
            "wot": wtile(Wo[:, e0:e0 + E].T, D),
            "lng": np.log(gvals)[None, :].astype(np.float32),
        })
    return in_maps


_cached = {}


def kernel(q, k, v, Wq, Wk, Wv, Wo, g, _trace=False, _tmpdir=None):
    if "nc" not in _cached:
        _cached["nc"] = build_kernel()
    nc = _cached["nc"]
    in_maps = make_in_maps(
        np.asarray(q, np.float32), np.asarray(k, np.float32),
        np.asarray(v, np.float32), np.asarray(Wq, np.float32),
        np.asarray(Wk, np.float32), np.asarray(Wv, np.float32),
        np.asarray(Wo, np.float32), g)
    res = run_bass_kernel_spmd(
        nc, in_maps, list(range(8)), trace=_trace, tmpdir=_tmpdir)
    out = np.empty((B, T, D), dtype=np.float32)
    for b in range(B):
        acc = res.results[4 * b]["outT"].copy()
        for gi in range(1, KVH):
            acc += res.results[4 * b + gi]["outT"]
        out[b] = acc.T
    kernel.last_results = res
    return out
